# revision 1
# baseline (speedup 1.0000x reference)
"""LSTM (T=512, B=64, D=H=1024) on 8 TRN2 NeuronCores.

Sharding: data-parallel over batch (8 batch elements per core), weights
replicated. Sequence stays local (recurrence is serial).

Per core:
  phase 1: x_comp = x @ Wi^T + (bi+bh), via bf16 GEMM with an augmented
           ones-row on x / bias-row on Wi^T; result f32 to DRAM scratch.
  phase 2: 512 serial steps; pre = x_comp[t] + h_{t-1} @ Wh^T (bf16 matmul,
           f32 PSUM accumulation), gates = sigmoid/tanh, cell/hidden update
           in f32, PE-transpose of h into bf16 lhsT layout for the next step.

Self-contained: shapes hardcoded; host code shards inputs, runs the SPMD
kernel via run_bass_kernel_spmd, and reassembles full outputs.
"""
import numpy as np
import ml_dtypes

T, B, D, H = 512, 64, 1024, 1024
G = 4 * H            # gate dim 4096
NC = 8               # cores
BL = B // NC         # batch per core = 8
KX = 9 * 128         # D padded with ones/bias row to 9 K-chunks
NK_D = 9             # K chunks for input GEMM
NK_H = 8             # K chunks for recurrent GEMM
NN = G // 512        # 8 N-chunks of 512

_BF16 = ml_dtypes.bfloat16

_compiled = None  # cache of the compiled Bass module


def _build():
    import concourse.tile as tile
    from concourse import bacc, mybir
    from concourse.bass import ds

    f32 = mybir.dt.float32
    bf16 = mybir.dt.bfloat16
    Sig = mybir.ActivationFunctionType.Sigmoid
    Tanh = mybir.ActivationFunctionType.Tanh

    nc = bacc.Bacc("TRN2", target_bir_lowering=False, debug=False, num_devices=NC)

    xT_d = nc.dram_tensor("xT", [KX, T * BL], bf16, kind="ExternalInput").ap()
    wiT_d = nc.dram_tensor("wiT", [KX, G], bf16, kind="ExternalInput").ap()
    whT_d = nc.dram_tensor("whT", [H, G], bf16, kind="ExternalInput").ap()
    h0T_d = nc.dram_tensor("h0T", [128, NK_H * BL], bf16, kind="ExternalInput").ap()
    c0_d = nc.dram_tensor("c0", [BL, H], f32, kind="ExternalInput").ap()
    id8_d = nc.dram_tensor("id8", [BL, BL], f32, kind="ExternalInput").ap()
    hs_d = nc.dram_tensor("hs", [T * BL, H], f32, kind="ExternalOutput").ap()
    c_out_d = nc.dram_tensor("c_out", [BL, H], f32, kind="ExternalOutput").ap()
    xcomp_d = nc.dram_tensor("xcomp", [T * BL, G], f32).ap()  # internal scratch

    # persistent state in SBUF (fixed addresses)
    hT_bf = nc.alloc_sbuf_tensor("hT_bf", [128, NK_H * BL], bf16).ap()
    c_sb = nc.alloc_sbuf_tensor("c_sb", [BL, H], f32).ap()
    id8_sb = nc.alloc_sbuf_tensor("id8_sb", [BL, BL], f32).ap()

    with tile.TileContext(nc) as tc:
        # ---------------- phase 1: input GEMM -> xcomp_d ----------------
        with tc.tile_pool(name="wiT", bufs=1) as wiT_pool, \
             tc.tile_pool(name="xt", bufs=2 * NK_D) as xt_pool, \
             tc.tile_pool(name="p1psum", bufs=2, space="PSUM") as p1_psum, \
             tc.tile_pool(name="p1out", bufs=3) as p1_out:
            wiT_sb = wiT_pool.tile([128, NK_D * G], bf16)
            for k in range(NK_D):
                nc.sync.dma_start(wiT_sb[:, k * G:(k + 1) * G],
                                  wiT_d[k * 128:(k + 1) * 128, :])
            for m in range(T * BL // 128):  # 32 tiles of 128 (t,b) rows
                xts = []
                for k in range(NK_D):
                    xt = xt_pool.tile([128, 128], bf16, tag="xt")
                    nc.sync.dma_start(
                        xt[:], xT_d[k * 128:(k + 1) * 128, m * 128:(m + 1) * 128])
                    xts.append(xt)
                for n in range(NN):
                    ps = p1_psum.tile([128, 512], f32, tag="p1ps")
                    for k in range(NK_D):
                        nc.tensor.matmul(
                            ps[:], xts[k][:],
                            wiT_sb[:, k * G + n * 512: k * G + n * 512 + 512],
                            start=(k == 0), stop=(k == NK_D - 1))
                    ot = p1_out.tile([128, 512], f32, tag="p1o")
                    nc.vector.tensor_copy(ot[:], ps[:])
                    nc.sync.dma_start(
                        xcomp_d[m * 128:(m + 1) * 128, n * 512:(n + 1) * 512], ot[:])

        # ---------------- phase 2: recurrence ----------------
        with tc.tile_pool(name="whT", bufs=1) as whT_pool, \
             tc.tile_pool(name="xc", bufs=2) as xc_pool, \
             tc.tile_pool(name="presb", bufs=4) as pre_pool, \
             tc.tile_pool(name="gates", bufs=2) as gates_pool, \
             tc.tile_pool(name="hc", bufs=2) as hc_pool, \
             tc.tile_pool(name="p2psum", bufs=4, space="PSUM") as p2_psum, \
             tc.tile_pool(name="trpsum", bufs=2, space="PSUM") as tr_psum:
            whT_sb = whT_pool.tile([128, NK_H * G], bf16)
            for k in range(NK_H):
                nc.sync.dma_start(whT_sb[:, k * G:(k + 1) * G],
                                  whT_d[k * 128:(k + 1) * 128, :])
            nc.sync.dma_start(hT_bf[:], h0T_d[:])
            nc.sync.dma_start(c_sb[:], c0_d[:])
            nc.sync.dma_start(id8_sb[:], id8_d[:])

            # process f,i,g chunks first so the cell update can start before o
            NORDER = [0, 1, 2, 3, 6, 7, 4, 5]

            with tc.For_i(0, T) as t:
                xc = xc_pool.tile([BL, G], f32, tag="xc")
                nc.sync.dma_start(xc[:], xcomp_d[ds(t * BL, BL), :])
                gates = gates_pool.tile([BL, G], f32, tag="gates")
                for n in NORDER:
                    ps = p2_psum.tile([BL, 512], f32, tag="pre")
                    for k in range(NK_H):
                        nc.tensor.matmul(
                            ps[:], hT_bf[:, k * BL:(k + 1) * BL],
                            whT_sb[:, k * G + n * 512: k * G + n * 512 + 512],
                            start=(k == 0), stop=(k == NK_H - 1))
                    pre = pre_pool.tile([BL, 512], f32, tag="pre_sb")
                    nc.vector.tensor_add(pre[:], ps[:], xc[:, n * 512:(n + 1) * 512])
                    nc.scalar.activation(gates[:, n * 512:(n + 1) * 512], pre[:],
                                         Tanh if n >= 6 else Sig)
                # c = f*c + i*g ; h = o*tanh(c)
                ig = hc_pool.tile([BL, H], f32, tag="ig")
                nc.vector.tensor_mul(ig[:], gates[:, H:2 * H], gates[:, 3 * H:4 * H])
                fc = hc_pool.tile([BL, H], f32, tag="fc")
                nc.vector.tensor_mul(fc[:], gates[:, 0:H], c_sb[:])
                nc.vector.tensor_add(c_sb[:], fc[:], ig[:])
                tanh_c = hc_pool.tile([BL, H], f32, tag="tanh_c")
                nc.scalar.activation(tanh_c[:], c_sb[:], Tanh)
                h = hc_pool.tile([BL, H], f32, tag="h")
                nc.vector.tensor_mul(h[:], gates[:, 2 * H:3 * H], tanh_c[:])
                nc.sync.dma_start(hs_d[ds(t * BL, BL), :], h[:])
                # transpose h into bf16 lhsT layout for the next step
                for k in range(NK_H):
                    trp = tr_psum.tile([128, BL], f32, tag="tr")
                    nc.tensor.transpose(trp[:], h[:, k * 128:(k + 1) * 128], id8_sb[:])
                    nc.vector.tensor_copy(hT_bf[:, k * BL:(k + 1) * BL], trp[:])
            nc.sync.dma_start(c_out_d[:], c_sb[:])

    nc.compile()
    return nc


def _get_compiled():
    global _compiled
    if _compiled is None:
        _compiled = _build()
    return _compiled


def make_in_maps(inputs, h0, c0, Wi, bi, Wh, bh):
    """Host-side sharding/layout prep. Returns list of per-core input dicts."""
    inputs = np.asarray(inputs, np.float32)
    h0 = np.asarray(h0, np.float32)
    c0 = np.asarray(c0, np.float32)
    Wi = np.asarray(Wi, np.float32)
    bi = np.asarray(bi, np.float32)
    Wh = np.asarray(Wh, np.float32)
    bh = np.asarray(bh, np.float32)

    wiT = np.zeros((KX, G), np.float32)
    wiT[:D] = Wi.T
    wiT[D] = bi + bh
    wiT_bf = wiT.astype(_BF16)
    whT_bf = Wh.T.astype(_BF16)
    id8 = np.eye(BL, dtype=np.float32)

    in_maps = []
    for c in range(NC):
        bs = slice(c * BL, (c + 1) * BL)
        xs = inputs[:, bs, :]                       # [T, BL, D]
        xT = np.zeros((KX, T * BL), np.float32)
        xT[:D] = xs.transpose(2, 0, 1).reshape(D, T * BL)
        xT[D] = 1.0
        h0s = h0[0, bs, :]                          # [BL, H]
        # hT[p, k*BL+b] = h0s[b, k*128+p]
        h0T = h0s.T.reshape(NK_H, 128, BL).transpose(1, 0, 2).reshape(128, NK_H * BL)
        in_maps.append({
            "xT": xT.astype(_BF16),
            "wiT": wiT_bf,
            "whT": whT_bf,
            "h0T": h0T.astype(_BF16),
            "c0": np.ascontiguousarray(c0[0, bs, :]),
            "id8": id8,
        })
    return in_maps


def run_compiled(in_maps, **kwargs):
    from concourse.bass_utils import run_bass_kernel_spmd
    nc = _get_compiled()
    return run_bass_kernel_spmd(nc, in_maps, core_ids=list(range(NC)), **kwargs)


def assemble(results):
    hs = np.empty((T, B, H), np.float32)
    c = np.empty((1, B, H), np.float32)
    for k in range(NC):
        bs = slice(k * BL, (k + 1) * BL)
        hs[:, bs, :] = results[k]["hs"].reshape(T, BL, H)
        c[0, bs, :] = results[k]["c_out"]
    h = hs[T - 1:T].copy()
    return hs, h, c


def kernel(inputs, h0, c0, Wi, bi, Wh, bh):
    in_maps = make_in_maps(inputs, h0, c0, Wi, bi, Wh, bh)
    res = run_compiled(in_maps)
    return assemble(res.results)
